# revision 1
# baseline (speedup 1.0000x reference)
"""Trainium2 Bass kernel for nn_Decoder_22196390985918 (SPADE-style decoder).

Sharding: 8 cores = (batch b in 0..3) x (H-half in 0..1). Each core computes
out[b, :, h0:h0+64, :] for h0 = 64*(core%2).

Key algorithmic transform: the [B, 512, H, W] "middle" tensor (masked scatter
of per-region style vectors mu[b,j,:]) is never materialized. Since
middle[b,:,h,w] = mu[b, j*(h,w), :] with j* the last active region,
conv(middle) collapses to a conv over the 5 one-hot region masks sel_j with
per-batch tap tables G[j, cc, tap] = sum_k Wconv[cc, k, tap] * mu[b, j, k].
That turns ~77 GFLOP of 512-channel convs into one K=45 matmul per tile.

The SPADE branch (mask -> shared 3x3 conv -> relu -> gamma/beta 3x3 convs) is
computed directly: shared conv via K=27 im2col, gamma/beta convs as 9
accumulating K=128 taps with gamma and beta fused into one M=128 output.
The sigmoid blending factors are folded into the conv weights and biases.

All conv/table matmuls run in float32r (TF32-like); everything else is fp32.
Each im2col is built by a single multi-dim-AP DMA per output chunk; DMA
issue is spread across the sync/tensor/scalar/gpsimd queues.
"""
import os as _os

import numpy as np

import concourse.bacc as bacc
import concourse.bass as bass
import concourse.mybir as mybir
import concourse.tile as tile
from concourse.bass_utils import run_bass_kernel_spmd

dt = mybir.dt
F32 = dt.float32
F32R = dt.float32 if _os.environ.get("KF32") == "1" else dt.float32r
AF = mybir.ActivationFunctionType
ALU = mybir.AluOpType

B, C, H, W, F, L, NH = 4, 64, 128, 128, 5, 512, 128
GW = 130                    # padded grid width  (image col = grid col - 1)
SR = 66                     # seg/sel/actv grid rows (image row = h0 - 1 + r)
MR = 68                     # mask grid rows (image row = h0 - 2 + r)
SEG_N = SR * GW             # 8580
MASK_N = MR * GW            # 8840
SEG_SZ = SEG_N + 2 * GW + 2 + 520   # sel tail slack for im2col windows
MASK_SZ = MASK_N + 2 * GW + 2 + 390
ROWS = 64                   # output rows per core
NCH = 16                    # main conv chunks (4 rows x 128 cols, N=512)
ACH = 22                    # shared conv chunks (3 rows x 128 cols, N=384)
NCORES = 8


def _win_ap(base_ap, flat):
    """9-tap im2col source view: partitions from base_ap, free dims
    (ty[3] x tx[3] x flat window) as overlapping strided windows."""
    return bass.AP(tensor=base_ap.tensor, offset=base_ap.offset,
                   ap=[base_ap.ap[0], [GW, 3], [1, 3], [1, flat]])


def _build_nc():
    lvl = int(_os.environ.get("KSEC", "8"))
    nc = bacc.Bacc()

    # ---- per-core DRAM inputs -------------------------------------------
    xb = nc.dram_tensor("xb", [C, H * W], F32, kind="ExternalInput")
    xown = nc.dram_tensor("xown", [C, ROWS * W], F32, kind="ExternalInput")
    segg = nc.dram_tensor("segg", [F, SEG_N + 264], F32, kind="ExternalInput")
    maskg = nc.dram_tensor("maskg", [3, MASK_N + 264], F32, kind="ExternalInput")
    codes = nc.dram_tensor("codes", [F, L], F32, kind="ExternalInput")
    fcw = nc.dram_tensor("fcw", [F, L, L], F32, kind="ExternalInput")
    fcbt = nc.dram_tensor("fcbt", [L, F], F32, kind="ExternalInput")
    cgw = nc.dram_tensor("cgw", [C, L * 9], F32, kind="ExternalInput")
    cbw = nc.dram_tensor("cbw", [C, L * 9], F32, kind="ExternalInput")
    sgw = nc.dram_tensor("sgw", [C, NH * 9], F32, kind="ExternalInput")
    sbw = nc.dram_tensor("sbw", [C, NH * 9], F32, kind="ExternalInput")
    ssw = nc.dram_tensor("ssw", [NH, 27], F32, kind="ExternalInput")
    cgb = nc.dram_tensor("cgb", [C, 1], F32, kind="ExternalInput")
    cbb = nc.dram_tensor("cbb", [C, 1], F32, kind="ExternalInput")
    sgbb = nc.dram_tensor("sgbb", [C, 1], F32, kind="ExternalInput")
    sbbb = nc.dram_tensor("sbbb", [C, 1], F32, kind="ExternalInput")
    ssb = nc.dram_tensor("ssb", [NH, 1], F32, kind="ExternalInput")
    bg = nc.dram_tensor("bg", [1, 1], F32, kind="ExternalInput")
    bb = nc.dram_tensor("bb", [1, 1], F32, kind="ExternalInput")
    u5 = nc.dram_tensor("u5", [45, 45], F32, kind="ExternalInput")
    ident = nc.dram_tensor("ident", [128, 128], F32, kind="ExternalInput")
    zz = nc.dram_tensor("zz", [128, 652], F32, kind="ExternalInput")
    hal = nc.dram_tensor("hal", [128, 2], F32, kind="ExternalInput")
    out_d = nc.dram_tensor("out", [C, NCH, 512], F32, kind="ExternalOutput")

    with tile.TileContext(nc) as tc:
        with (
            tc.tile_pool(name="const", bufs=1) as cst,
            tc.tile_pool(name="wcb", bufs=4) as wcbp,
            tc.tile_pool(name="wct", bufs=4) as wctp,
            tc.tile_pool(name="fcwp", bufs=2) as fcwp,
            tc.tile_pool(name="cbcp", bufs=1) as cbcp,
            tc.tile_pool(name="ttp", bufs=2) as ttp,
            tc.tile_pool(name="xs", bufs=2) as xsp,
            tc.tile_pool(name="gb", bufs=2) as gbp,
            tc.tile_pool(name="xn", bufs=2) as xnp,
            tc.tile_pool(name="ot", bufs=2) as otp,
            tc.tile_pool(name="pmain", bufs=2, space="PSUM") as pmain,
            tc.tile_pool(name="paux", bufs=2, space="PSUM") as paux,
            tc.tile_pool(name="gpsp", bufs=3, space="PSUM") as gpsp,
        ):
            # ---- tiny constants (sync queue head) -----------------------
            u5r = cst.tile([45, 45], F32R)
            nc.sync.dma_start(out=u5r[:], in_=u5[:].bitcast(F32R))
            id_t = cst.tile([128, 128], F32)
            nc.sync.dma_start(out=id_t[:], in_=ident[:])
            id_r = cst.tile([128, 128], F32R)
            nc.sync.dma_start(out=id_r[:], in_=ident[:].bitcast(F32R))
            sswf = cst.tile([NH, 27], F32)
            nc.sync.dma_start(out=sswf[:], in_=ssw[:])
            graw = cst.tile([128, 1], F32)
            nc.sync.dma_start(out=graw[:], in_=bg[:].to_broadcast((128, 1)))
            braw = cst.tile([128, 1], F32)
            nc.sync.dma_start(out=braw[:], in_=bb[:].to_broadcast((128, 1)))
            convb = cst.tile([128, 1], F32)
            nc.sync.dma_start(out=convb[0:64, :], in_=cgb[:])
            nc.sync.dma_start(out=convb[64:128, :], in_=cbb[:])
            spadeb = cst.tile([128, 1], F32)
            nc.sync.dma_start(out=spadeb[0:64, :], in_=sgbb[:])
            nc.sync.dma_start(out=spadeb[64:128, :], in_=sbbb[:])
            ssb_t = cst.tile([NH, 1], F32)
            nc.sync.dma_start(out=ssb_t[:], in_=ssb[:])
            hal_t = cst.tile([128, 2], F32)
            nc.sync.dma_start(out=hal_t[:], in_=hal[:])
            fcbt_sb = cst.tile([128, 4, F], F32)
            for kb in range(4):
                nc.sync.dma_start(out=fcbt_sb[:, kb, :],
                                  in_=fcbt[kb * 128:(kb + 1) * 128, :])

            ones_t = cst.tile([128, 1], F32)
            nc.gpsimd.memset(ones_t[:], 1.0)
            eps_t = cst.tile([C, 1], F32)
            nc.gpsimd.memset(eps_t[:], 1e-5)
            half1 = cst.tile([128, 1], F32)
            nc.gpsimd.memset(half1[0:64, :], 1.0)
            nc.gpsimd.memset(half1[64:128, :], 0.0)
            zsb = cst.tile([128, 132], F32)
            nc.gpsimd.memset(zsb[:], 0.0)

            # blending factors
            gsig = cst.tile([128, 1], F32)
            nc.scalar.activation(gsig[:], graw[:], AF.Sigmoid)
            bsig = cst.tile([128, 1], F32)
            nc.scalar.activation(bsig[:], braw[:], AF.Sigmoid)
            gba = cst.tile([128, 1], F32)
            nc.vector.tensor_copy(gba[0:64, :], gsig[0:64, :])
            nc.vector.tensor_copy(gba[64:128, :], bsig[64:128, :])
            om_gba = cst.tile([128, 1], F32)
            nc.scalar.activation(om_gba[:], gba[:], AF.Identity, bias=ones_t[:], scale=-1.0)
            tb1 = cst.tile([128, 1], F32)
            nc.vector.tensor_mul(tb1[:], convb[:], gba[:])
            tb2 = cst.tile([128, 1], F32)
            nc.vector.tensor_mul(tb2[:], spadeb[:], om_gba[:])
            bias_t = cst.tile([128, 1], F32)
            nc.vector.tensor_add(bias_t[:], tb1[:], tb2[:])
            bias1_t = cst.tile([128, 1], F32)
            nc.vector.tensor_add(bias1_t[:], bias_t[:], half1[:])

            # ---- big weight loads: wcb (PE-critical) then fw (scalar q) -
            wcbs = []
            for kb in range(4):
                wcb = wcbp.tile([128, 1152], F32, tag="wcb", name=f"wcb{kb}")
                nc.sync.dma_start(out=wcb[0:64, :], in_=cgw[:, kb * 1152:(kb + 1) * 1152])
                nc.sync.dma_start(out=wcb[64:128, :], in_=cbw[:, kb * 1152:(kb + 1) * 1152])
                wcbs.append(wcb)
            sgb = cst.tile([128, 1152], F32)
            nc.sync.dma_start(out=sgb[0:64, :], in_=sgw[:])
            nc.sync.dma_start(out=sgb[64:128, :], in_=sbw[:])

            # ---- grids: pre-shifted replicated loads (gpsimd queue) -----
            sel45 = cst.tile([45, SEG_N], F32R)
            segp = segg[:].ap[0][0]
            for ty in range(3):
                src = bass.AP(tensor=segg[:].tensor, offset=ty * GW,
                              ap=[[1, 3], [segp, F], [1, SEG_N]])
                nc.gpsimd.dma_start(out=sel45[15 * ty:15 * ty + 15, :],
                                    in_=src.bitcast(F32R))
            mask27 = cst.tile([27, MASK_N], F32R)
            maskp_ = maskg[:].ap[0][0]
            for ty in range(3):
                src = bass.AP(tensor=maskg[:].tensor, offset=ty * GW,
                              ap=[[1, 3], [maskp_, 3], [1, MASK_N]])
                nc.gpsimd.dma_start(out=mask27[9 * ty:9 * ty + 9, :],
                                    in_=src.bitcast(F32R))

            # ---- region masks part 1: cnt (PE) -> t (ACT, parked in SBUF)
            t_sb = cst.tile([45, SEG_N], dt.bfloat16)
            segchunks = []
            off = 0 if lvl >= 2 else SEG_N
            while off < SEG_N:
                n = min(512, SEG_N - off)
                segchunks.append((off, n))
                off += n
            for off, n in segchunks:
                pc = paux.tile([45, 512], F32, tag="aux")
                nc.tensor.matmul(pc[:, 0:n], u5r[:], sel45[:, off:off + n],
                                 start=True, stop=True)
                nc.scalar.activation(t_sb[:, off:off + n], pc[:, 0:n], AF.Relu,
                                     bias=ones_t[0:45, :], scale=-1.0)

            # ---- shared conv (mask 3 -> NH), pre-shifted rows -----------
            if lvl >= 3:
                ptp = paux.tile([27, 128], F32, tag="aux")
                nc.tensor.transpose(ptp[:], sswf[:], id_t[:])
                sswT = cst.tile([27, 128], F32R)
                nc.scalar.activation(sswT[:], ptp[:], AF.Copy)

                actv = cst.tile([NH, SR, GW], F32R)
                bord = actv[:, :, 0:1]
                nc.vector.tensor_copy(
                    bass.AP(tensor=bord.tensor, offset=bord.offset,
                            ap=[bord.ap[0], [GW, SR], [GW - 1, 2]]),
                    zsb[:].rearrange("p (a b) -> p a b", a=SR))
                m3 = mask27[:].rearrange("p (r c) -> p r c", c=GW)
                for a in range(ACH):
                    r = 3 * a
                    psh = paux.tile([NH, 3, 128], F32, tag="aux")
                    nc.tensor.matmul(psh[:], sswT[:], m3[:, r:r + 3, 0:128],
                                     start=True, stop=True)
                    nc.scalar.activation(actv[:, r:r + 3, 1:129], psh[:], AF.Relu,
                                         bias=ssb_t[:], scale=1.0)

            # ---- spade gamma/beta lhsT ----------------------------------
            if lvl >= 6:
                nc.vector.tensor_scalar_mul(sgb[:], sgb[:], om_gba[:])
                spT = cst.tile([128, 9, 128], F32R)
                sgb3 = sgb[:].rearrange("p (l t) -> p l t", t=9)
                for t in range(9):
                    pt = paux.tile([128, 128], F32, tag="aux")
                    nc.tensor.transpose(pt[:], sgb3[:, :, t], id_t[:])
                    nc.scalar.activation(spT[:, t, :], pt[:], AF.Copy)

            # ---- mu path (fw on scalar queue; muls split DVE/Pool) ------
            if lvl >= 4:
                z_sb = cst.tile([128, 4, F], F32)
                muT = cst.tile([128, 4, F], F32R)
                for j in range(F):
                    cbc = cbcp.tile([128, L], F32, tag="cbc")
                    nc.scalar.dma_start(out=cbc[:],
                                        in_=codes[j:j + 1, :].to_broadcast((128, L)))
                    eng = nc.vector if j < 3 else nc.gpsimd
                    for kb in range(4):
                        fw = fcwp.tile([128, L], F32, tag="fcw")
                        nc.scalar.dma_start(out=fw[:], in_=fcw[j, kb * 128:(kb + 1) * 128, :])
                        tts = ttp.tile([128, L], F32, tag="tts")
                        eng.tensor_mul(tts[:], fw[:], cbc[:])
                        nc.vector.reduce_sum(out=z_sb[:, kb, j:j + 1], in_=tts[:],
                                             axis=mybir.AxisListType.X)
                for kb in range(4):
                    nc.vector.tensor_add(z_sb[:, kb, :], z_sb[:, kb, :],
                                         fcbt_sb[:, kb, :])
                for kb in range(4):
                    nc.scalar.activation(muT[:, kb, :], z_sb[:, kb, :], AF.Relu)

            # ---- wct transposes (PE) + copies (ACT) ---------------------
            if lvl >= 5:
                wcts = []
                for kb in range(4):
                    wcb = wcbs[kb]
                    nc.vector.tensor_scalar_mul(wcb[:], wcb[:], gba[:])
                    wct = wctp.tile([128, 9, 128], F32R, tag="wct", name=f"wct{kb}")
                    wcb3 = wcb[:].rearrange("p (l t) -> p l t", t=9)
                    for t in range(9):
                        pt = paux.tile([128, 128], F32, tag="aux")
                        nc.tensor.transpose(pt[:], wcb3[:, :, t], id_t[:])
                        nc.scalar.activation(wct[:, t, :], pt[:], AF.Copy)
                    wcts.append(wct)

            # ---- region masks part 2: sel = seg * t (DVE, after mu) -----
            for off, n in segchunks:
                nc.vector.tensor_mul(sel45[:, off:off + n],
                                     sel45[:, off:off + n].bitcast(F32),
                                     t_sb[:, off:off + n])
            if lvl >= 3:
                nc.vector.tensor_scalar_mul(actv[:, 0, :], actv[:, 0, :].bitcast(F32),
                                            hal_t[:, 0:1])
                nc.vector.tensor_scalar_mul(actv[:, SR - 1, :], actv[:, SR - 1, :].bitcast(F32),
                                            hal_t[:, 1:2])

            # ---- G matmuls -> selG --------------------------------------
            if lvl >= 5:
                gps = [gpsp.tile([F, 3, 128], F32, tag="gps", name=f"gps{_g}")
                       for _g in range(3)]
                for kb in range(4):
                    for g in range(3):
                        nc.tensor.matmul(gps[g][:], muT[:, kb, :],
                                         wcts[kb][:, 3 * g:3 * g + 3, :],
                                         start=(kb == 0), stop=(kb == 3))
                selG = cst.tile([45, 128], F32R)
                gstage = cst.tile([F, 9, 128], F32)
                for g in range(3):
                    nc.scalar.activation(gstage[:, 3 * g:3 * g + 3, :], gps[g][:], AF.Copy)
                for t in range(9):
                    nc.sync.dma_start(out=selG[F * t:F * t + F, :],
                                      in_=gstage[:, t, :].bitcast(F32R))

            # ---- instance-norm stats (xb on sync queue) -----------------
            if lvl >= 7:
                stats_t = cst.tile([C, 32, 6], F32)
                for q in range(16):
                    xt = xsp.tile([C, 2, 512], F32, tag="xs")
                    nc.sync.dma_start(out=xt[:], in_=xb[:, q * 1024:(q + 1) * 1024]
                                      .rearrange("c (k n) -> c k n", k=2))
                    for k in range(2):
                        nc.vector.bn_stats(out=stats_t[:, 2 * q + k, :], in_=xt[:, k, :])
                mv = cst.tile([C, 2], F32)
                nc.vector.bn_aggr(out=mv[:], in_=stats_t[:])
                sd = cst.tile([C, 1], F32)
                nc.scalar.activation(sd[:], mv[:, 1:2], AF.Sqrt, bias=eps_t[:], scale=1.0)
                rstd = cst.tile([C, 1], F32)
                nc.vector.reciprocal(rstd[:], sd[:])
                nbias = cst.tile([C, 1], F32)
                nc.vector.tensor_mul(nbias[:], mv[:, 0:1], rstd[:])
                nc.vector.tensor_scalar_mul(nbias[:], nbias[:], -1.0)

            # ---- main conv + epilogue (epilogue one chunk behind) -------
            if lvl >= 8:
                s3 = sel45[:].rearrange("p (r c) -> p r c", c=GW)
                xt2s, xnts, pms = {}, {}, {}

                def conv_chunk(i):
                    xt2s[i] = xnp.tile([C, 4, 128], F32, tag="xn", name=f"xt2_{i}")
                    nc.gpsimd.dma_start(out=xt2s[i][:],
                                        in_=xown[:, i * 512:(i + 1) * 512].rearrange(
                                            "c (r w) -> c r w", r=4))
                    xnts[i] = otp.tile([C, 4, 128], F32, tag="ot", name=f"xnt_{i}")
                    pm = pmain.tile([128, 4, 128], F32, tag="pm", name=f"pm_{i}")
                    pms[i] = pm
                    for t in range(9):
                        ty, tx = divmod(t, 3)
                        nc.tensor.matmul(pm[:], spT[:, t, :],
                                         actv[:, 4 * i + ty:4 * i + ty + 4, tx:tx + 128],
                                         start=(t == 0), stop=False)
                    nc.tensor.matmul(pm[:], selG[:], s3[:, 4 * i:4 * i + 4, 0:128],
                                     start=False, stop=True)

                def epi_chunk(i):
                    pm = pms.pop(i)
                    gb = gbp.tile([128, 4, 128], F32R, tag="gb", name=f"gb_{i}")
                    nc.scalar.activation(gb[:], pm[:], AF.Identity,
                                         bias=bias1_t[:], scale=1.0)
                    pb = gpsp.tile([64, 4, 128], F32, tag="gps", name=f"pb_{i}")
                    nc.tensor.matmul(pb[:].rearrange("p t c -> p (t c)"), id_r[:, 64:128],
                                     gb[:].rearrange("p t c -> p (t c)"),
                                     start=True, stop=True)
                    xt2, xnt = xt2s.pop(i), xnts[i]
                    nc.gpsimd.tensor_scalar(xnt[:], xt2[:],
                                            rstd[:], nbias[:],
                                            op0=ALU.mult, op1=ALU.add)
                    nc.gpsimd.tensor_mul(xnt[:], xnt[:], gb[0:64, :, :].bitcast(F32))
                    nc.vector.tensor_add(xnt[:].rearrange("p t c -> p (t c)"),
                                         xnt[:].rearrange("p t c -> p (t c)"),
                                         pb[:].rearrange("p t c -> p (t c)"))
                    nc.sync.dma_start(out=out_d[:, i, :],
                                      in_=xnts.pop(i)[:].rearrange("c r w -> c (r w)"))

                conv_chunk(0)
                for i in range(1, NCH):
                    conv_chunk(i)
                    epi_chunk(i - 1)
                epi_chunk(NCH - 1)

    nc.finalize()
    return nc


_NC = None


def kernel(**inputs):
    global _NC
    x = np.asarray(inputs["x"], dtype=np.float32)
    segmap = np.asarray(inputs["segmap"], dtype=np.float32)
    codes_vector = np.asarray(inputs["codes_vector"], dtype=np.float32)
    mask = np.asarray(inputs["mask"], dtype=np.float32)
    fc_w = np.ascontiguousarray(np.asarray(inputs["fc_w"], dtype=np.float32))
    fc_b = np.asarray(inputs["fc_b"], dtype=np.float32)
    conv_gamma_w = np.asarray(inputs["conv_gamma_w"], dtype=np.float32)
    conv_gamma_b = np.asarray(inputs["conv_gamma_b"], dtype=np.float32)
    conv_beta_w = np.asarray(inputs["conv_beta_w"], dtype=np.float32)
    conv_beta_b = np.asarray(inputs["conv_beta_b"], dtype=np.float32)
    spade_shared_w = np.asarray(inputs["spade_shared_w"], dtype=np.float32)
    spade_shared_b = np.asarray(inputs["spade_shared_b"], dtype=np.float32)
    spade_gamma_w = np.asarray(inputs["spade_gamma_w"], dtype=np.float32)
    spade_gamma_b = np.asarray(inputs["spade_gamma_b"], dtype=np.float32)
    spade_beta_w = np.asarray(inputs["spade_beta_w"], dtype=np.float32)
    spade_beta_b = np.asarray(inputs["spade_beta_b"], dtype=np.float32)
    blending_gamma = np.asarray(inputs["blending_gamma"], dtype=np.float32)
    blending_beta = np.asarray(inputs["blending_beta"], dtype=np.float32)

    if _NC is None:
        _NC = _build_nc()

    shared = {
        "fcw": np.ascontiguousarray(fc_w),
        "fcbt": np.ascontiguousarray(fc_b.T),
        "cgw": np.ascontiguousarray(conv_gamma_w.reshape(C, L * 9)),
        "cbw": np.ascontiguousarray(conv_beta_w.reshape(C, L * 9)),
        "sgw": np.ascontiguousarray(spade_gamma_w.reshape(C, NH * 9)),
        "sbw": np.ascontiguousarray(spade_beta_w.reshape(C, NH * 9)),
        "ssw": np.ascontiguousarray(spade_shared_w.transpose(0, 2, 3, 1).reshape(NH, 27)),
        "cgb": conv_gamma_b.reshape(C, 1), "cbb": conv_beta_b.reshape(C, 1),
        "sgbb": spade_gamma_b.reshape(C, 1), "sbbb": spade_beta_b.reshape(C, 1),
        "ssb": spade_shared_b.reshape(NH, 1),
        "bg": blending_gamma.reshape(1, 1), "bb": blending_beta.reshape(1, 1),
        "u5": np.kron(np.eye(9, dtype=np.float32), np.tril(np.ones((F, F), np.float32), -1)),
        "ident": np.eye(128, dtype=np.float32),
        "zz": np.zeros((128, 652), np.float32),
    }

    in_maps = []
    for c in range(NCORES):
        b, half = divmod(c, 2)
        h0 = half * ROWS
        segp = np.zeros((F, SR * GW + 264), np.float32).reshape(F, -1)
        segp2 = np.zeros((F, SR, GW), np.float32)
        r_lo, r_hi = h0 - 1, h0 + ROWS + 1  # exclusive
        s_lo, s_hi = max(r_lo, 0), min(r_hi, H)
        segp2[:, s_lo - r_lo:s_hi - r_lo, 1:129] = segmap[b, :, s_lo:s_hi, :]
        segp[:, 0:SR * GW] = segp2.reshape(F, -1)
        maskp = np.zeros((3, MR * GW + 264), np.float32)
        maskp2 = np.zeros((3, MR, GW), np.float32)
        m_lo, m_hi = h0 - 2, h0 + ROWS + 2
        ms_lo, ms_hi = max(m_lo, 0), min(m_hi, H)
        maskp2[:, ms_lo - m_lo:ms_hi - m_lo, 1:129] = mask[b, :, ms_lo:ms_hi, :]
        maskp[:, 0:MR * GW] = maskp2.reshape(3, -1)
        in_maps.append(dict(
            shared,
            xb=np.ascontiguousarray(x[b].reshape(C, H * W)),
            xown=np.ascontiguousarray(x[b, :, h0:h0 + ROWS, :].reshape(C, ROWS * W)),
            hal=np.ones((128, 2), np.float32) * np.array(
                [0.0 if h0 == 0 else 1.0, 0.0 if h0 + ROWS == H else 1.0],
                np.float32)[None, :],
            segg=np.ascontiguousarray(segp),
            maskg=np.ascontiguousarray(maskp),
            codes=np.ascontiguousarray(codes_vector[b]),
        ))

    res = run_bass_kernel_spmd(_NC, in_maps, list(range(NCORES)))

    out = np.empty((B, C, H, W), np.float32)
    for c in range(NCORES):
        b, half = divmod(c, 2)
        h0 = half * ROWS
        out[b, :, h0:h0 + ROWS, :] = res.results[c]["out"].reshape(C, ROWS, W)
    return out



# revision 23
# speedup vs baseline: 1.5887x; 1.5887x over previous
"""Trainium2 Bass kernel for nn_Decoder_22196390985918 (SPADE-style decoder).

Sharding: 8 cores = (batch b in 0..3) x (H-half in 0..1). Each core computes
out[b, :, h0:h0+64, :] for h0 = 64*(core%2).

Key algorithmic transform: the [B, 512, H, W] "middle" tensor (masked scatter
of per-region style vectors mu[b,j,:]) is never materialized. Since
middle[b,:,h,w] = mu[b, j*(h,w), :] with j* the last active region,
conv(middle) collapses to a conv over the 5 one-hot region masks sel_j with
per-batch tap tables G[j, cc, tap] = sum_k Wconv[cc, k, tap] * mu[b, j, k].
That turns ~77 GFLOP of 512-channel convs into one K=45 matmul per tile.

v2 layout: all heavy tensors travel in bf16 (fp32 PSUM accumulation); the
fc Linear (mu) runs on the PE against host-pretransposed fc weights; the
region conv tap weights are host-pretransposed so no device transposes are
needed (the sigmoid gamma/beta blend factors are folded into two scaled
copies of mu^T instead); x is loaded once (the per-core 64-row slice is a
view of the full plane used for the instance-norm stats); the beta-half
partition shift runs as an SBUF->SBUF DMA instead of a PE matmul.
"""
import numpy as np
import ml_dtypes

import concourse.bacc as bacc
import concourse.bass as bass
import concourse.mybir as mybir
import concourse.tile as tile
from concourse.bass_utils import run_bass_kernel_spmd

dt = mybir.dt
F32 = dt.float32
BF16 = dt.bfloat16
AF = mybir.ActivationFunctionType
ALU = mybir.AluOpType
NPBF = ml_dtypes.bfloat16

B, C, H, W, F, L, NH = 4, 64, 128, 128, 5, 512, 128
GW = 130                    # padded grid width  (image col = grid col - 1)
SR = 66                     # seg/sel/actv grid rows (image row = h0 - 1 + r)
MR = 68                     # mask grid rows (image row = h0 - 2 + r)
SEG_N = SR * GW             # 8580
MASK_N = MR * GW            # 8840
SEG_SZ = SEG_N + 2 * GW + 2 + 520   # sel tail slack for im2col windows
MASK_SZ = MASK_N + 2 * GW + 2 + 390
ROWS = 64                   # output rows per core
NCH = 16                    # main conv chunks (4 rows x 128 cols, N=512)
ACH = 22                    # shared conv chunks (3 rows x 128 cols, N=384)
NCORES = 8


def _build_nc():
    nc = bacc.Bacc()

    # ---- per-core DRAM inputs -------------------------------------------
    xb = nc.dram_tensor("xb", [C, H * W], BF16, kind="ExternalInput")
    segg = nc.dram_tensor("segg", [F, SEG_SZ], BF16, kind="ExternalInput")
    maskg = nc.dram_tensor("maskg", [3, MASK_SZ], BF16, kind="ExternalInput")
    codes2 = nc.dram_tensor("codes2", [128, 4 * F * F], BF16,
                            kind="ExternalInput")
    fcwT = nc.dram_tensor("fcwT", [128, F * 4 * 512], BF16, kind="ExternalInput")
    wctd = nc.dram_tensor("wctd", [128, 4 * 9 * 128], BF16, kind="ExternalInput")
    sgbd = nc.dram_tensor("sgbd", [128, NH * 9], BF16, kind="ExternalInput")
    sswT = nc.dram_tensor("sswT", [27, NH], BF16, kind="ExternalInput")
    u5 = nc.dram_tensor("u5", [45, 45], BF16, kind="ExternalInput")
    ident = nc.dram_tensor("ident", [128, 128], BF16, kind="ExternalInput")
    fcb = nc.dram_tensor("fcb", [F, L], BF16, kind="ExternalInput")
    cgb = nc.dram_tensor("cgb", [C, 1], F32, kind="ExternalInput")
    cbb = nc.dram_tensor("cbb", [C, 1], F32, kind="ExternalInput")
    sgbb = nc.dram_tensor("sgbb", [C, 1], F32, kind="ExternalInput")
    sbbb = nc.dram_tensor("sbbb", [C, 1], F32, kind="ExternalInput")
    ssb = nc.dram_tensor("ssb", [NH, 1], F32, kind="ExternalInput")
    bg = nc.dram_tensor("bg", [1, 1], F32, kind="ExternalInput")
    bb = nc.dram_tensor("bb", [1, 1], F32, kind="ExternalInput")
    hal = nc.dram_tensor("hal", [128, 2], F32, kind="ExternalInput")
    out_d = nc.dram_tensor("out", [C, NCH, 512], F32, kind="ExternalOutput")
    import os as _os
    DBG = _os.environ.get("KDBG") == "1"
    if DBG:
        dbg_mur = nc.dram_tensor("dbg_mur", [F, L], BF16, kind="ExternalOutput")
        dbg_selG = nc.dram_tensor("dbg_selG", [45, 128], BF16, kind="ExternalOutput")
        dbg_actv = nc.dram_tensor("dbg_actv", [NH, 3 * GW], BF16, kind="ExternalOutput")
        dbg_spT = nc.dram_tensor("dbg_spT", [128, 2 * 128], BF16, kind="ExternalOutput")
        dbg_sel45 = nc.dram_tensor("dbg_sel45", [45, 512], BF16, kind="ExternalOutput")
        dbg_muTg = nc.dram_tensor("dbg_muTg", [128, 4 * F], BF16, kind="ExternalOutput")
        dbg_stats = nc.dram_tensor("dbg_stats", [C, 2], F32, kind="ExternalOutput")
        dbg_gb = nc.dram_tensor("dbg_gb", [128, 512], BF16, kind="ExternalOutput")
        dbg_wct = nc.dram_tensor("dbg_wct", [128, 9 * 128], BF16, kind="ExternalOutput")
        dbg_gst = nc.dram_tensor("dbg_gst", [F, 9 * 128], BF16, kind="ExternalOutput")

    with tile.TileContext(nc) as tc:
        with (
            tc.tile_pool(name="const", bufs=1) as cst,
            tc.tile_pool(name="gb", bufs=2) as gbp,
            tc.tile_pool(name="pbt", bufs=2) as pbtp,
            tc.tile_pool(name="xn", bufs=2) as xnp,
            tc.tile_pool(name="pmain", bufs=2, space="PSUM") as pmain,
            tc.tile_pool(name="paux", bufs=2, space="PSUM") as paux,
            tc.tile_pool(name="gpsp", bufs=3, space="PSUM") as gpsp,
            tc.tile_pool(name="pmu", bufs=1, space="PSUM") as pmu,
        ):
            # ---- big loads first, one queue each ------------------------
            # sync: x plane (stats + epilogue slices)
            xb_sb = cst.tile([C, H * W], BF16)
            for q in range(4):
                nc.sync.dma_start(out=xb_sb[:, q * 4096:(q + 1) * 4096],
                                  in_=xb[:, q * 4096:(q + 1) * 4096])
            # scalar: codes lhsT then fc weights, per region (mu starts early)
            # codes5[:, lc, j, :] is a [128, F] lhsT whose only nonzero column
            # is j (= codes[j, lc*128:+128]), so each region's matmul lands on
            # its own output partition while all 21 accumulate in one group.
            cT = cst.tile([128, 4, F, F], BF16)
            nc.scalar.dma_start(out=cT[:].rearrange("p a j k -> p (a j k)"),
                                in_=codes2[:])
            fw_sb = cst.tile([128, F, 4, 512], BF16)
            for j in range(F):
                nc.scalar.dma_start(
                    out=fw_sb[:, j, :, :].rearrange("p a k -> p (a k)"),
                    in_=fcwT[:, j * 2048:(j + 1) * 2048])
            # gpsimd: cnt weights then pre-shifted replicated grids
            u5r = cst.tile([45, 45], BF16)
            nc.gpsimd.dma_start(out=u5r[:], in_=u5[:])
            sswT_sb = cst.tile([27, NH], BF16)
            nc.gpsimd.dma_start(out=sswT_sb[:], in_=sswT[:])
            sel45 = cst.tile([45, SEG_N], BF16)
            segp = segg[:].ap[0][0]
            for ty in range(3):
                src = bass.AP(tensor=segg[:].tensor, offset=ty * GW,
                              ap=[[1, 3], [segp, F], [1, SEG_N]])
                nc.gpsimd.dma_start(out=sel45[15 * ty:15 * ty + 15, :], in_=src)
            mask27 = cst.tile([27, MASK_N], BF16)
            maskp_ = maskg[:].ap[0][0]
            for ty in range(3):
                src = bass.AP(tensor=maskg[:].tensor, offset=ty * GW,
                              ap=[[1, 3], [maskp_, 3], [1, MASK_N]])
                nc.gpsimd.dma_start(out=mask27[9 * ty:9 * ty + 9, :], in_=src)
            # gpsimd: region conv tap weights (host-transposed) + spade
            wct_sb = cst.tile([128, 4, 9, 128], BF16)
            nc.gpsimd.dma_start(
                out=wct_sb[:].rearrange("p a t c -> p (a t c)"), in_=wctd[:])
            sgb = cst.tile([128, NH * 9], BF16)
            nc.gpsimd.dma_start(out=sgb[:], in_=sgbd[:])

            # ---- small consts (sync queue) ------------------------------
            id_bf = cst.tile([128, 128], BF16)
            nc.sync.dma_start(out=id_bf[:], in_=ident[:])
            fcb_sb = cst.tile([F, L], BF16)
            nc.scalar.dma_start(out=fcb_sb[:], in_=fcb[:])
            graw = cst.tile([128, 1], F32)
            nc.sync.dma_start(out=graw[:], in_=bg[:].to_broadcast((128, 1)))
            braw = cst.tile([128, 1], F32)
            nc.sync.dma_start(out=braw[:], in_=bb[:].to_broadcast((128, 1)))
            convb = cst.tile([128, 1], F32)
            nc.sync.dma_start(out=convb[0:64, :], in_=cgb[:])
            nc.sync.dma_start(out=convb[64:128, :], in_=cbb[:])
            spadeb = cst.tile([128, 1], F32)
            nc.sync.dma_start(out=spadeb[0:64, :], in_=sgbb[:])
            nc.sync.dma_start(out=spadeb[64:128, :], in_=sbbb[:])
            ssb_t = cst.tile([NH, 1], F32)
            nc.sync.dma_start(out=ssb_t[:], in_=ssb[:])
            hal_t = cst.tile([128, 2], F32)
            nc.sync.dma_start(out=hal_t[:], in_=hal[:])

            ones_t = cst.tile([128, 1], F32)
            nc.gpsimd.memset(ones_t[:], 1.0)
            eps_t = cst.tile([C, 1], F32)
            nc.gpsimd.memset(eps_t[:], 1e-5)
            half1 = cst.tile([128, 1], F32)
            nc.gpsimd.memset(half1[0:64, :], 1.0)
            nc.gpsimd.memset(half1[64:128, :], 0.0)
            zsb = cst.tile([128, 132], BF16)
            nc.gpsimd.memset(zsb[:], 0.0)

            # blending factors (scalar queue, tiny)
            gsig = cst.tile([128, 1], F32)
            nc.scalar.activation(gsig[:], graw[:], AF.Sigmoid)
            bsig = cst.tile([128, 1], F32)
            nc.scalar.activation(bsig[:], braw[:], AF.Sigmoid)
            gba = cst.tile([128, 1], F32)
            nc.vector.tensor_copy(gba[0:64, :], gsig[0:64, :])
            nc.vector.tensor_copy(gba[64:128, :], bsig[64:128, :])
            om_gba = cst.tile([128, 1], F32)
            nc.scalar.activation(om_gba[:], gba[:], AF.Identity,
                                 bias=ones_t[:], scale=-1.0)
            tb1 = cst.tile([128, 1], F32)
            nc.vector.tensor_mul(tb1[:], convb[:], gba[:])
            tb2 = cst.tile([128, 1], F32)
            nc.vector.tensor_mul(tb2[:], spadeb[:], om_gba[:])
            bias_t = cst.tile([128, 1], F32)
            nc.vector.tensor_add(bias_t[:], tb1[:], tb2[:])
            bias1_t = cst.tile([128, 1], F32)
            nc.vector.tensor_add(bias1_t[:], bias_t[:], half1[:])
            ones128 = cst.tile([128, 128], F32)
            nc.gpsimd.memset(ones128[:], 1.0)
            blendT = cst.tile([128, 128], BF16)
            nc.scalar.activation(blendT[:, 0:64], ones128[:, 0:64], AF.Copy,
                                 scale=gsig[:])
            nc.scalar.activation(blendT[:, 64:128], ones128[:, 64:128],
                                 AF.Copy, scale=bsig[:])

            # ---- PE stream: mu matmuls (fc Linear, all regions, 1 group)
            mu_ps = pmu.tile([F, L], F32, tag="mu", name="mu_ps")
            nc.tensor.matmul(mu_ps[:], id_bf[0:F, 0:F], fcb_sb[:],
                             start=True, stop=False)
            for j in range(F):
                for lc in range(4):
                    nc.tensor.matmul(mu_ps[:], cT[:, lc, j, :],
                                     fw_sb[:, j, lc, :],
                                     start=False, stop=(j == F - 1 and lc == 3))

            # ---- PE stream: region count matmuls ------------------------
            t_sb = cst.tile([45, SEG_N], BF16)
            segchunks = []
            off = 0
            while off < SEG_N:
                n = min(512, SEG_N - off)
                segchunks.append((off, n))
                off += n
            for off, n in segchunks:
                pc = paux.tile([45, 512], F32, tag="aux")
                nc.tensor.matmul(pc[:, 0:n], u5r[:], sel45[:, off:off + n],
                                 start=True, stop=True)
                nc.scalar.activation(t_sb[:, off:off + n], pc[:, 0:n], AF.Relu,
                                     bias=ones_t[0:45, :], scale=-1.0)

            # ---- mu epilogue: relu, transpose, 2 scaled copies ----------
            mur = cst.tile([F, L], BF16)
            nc.scalar.activation(mur[:], mu_ps[:], AF.Relu)
            muT = cst.tile([128, 4, F], BF16)
            for kb in range(4):
                mt = paux.tile([128, F], BF16, tag="aux")
                nc.tensor.transpose(mt[:], mur[:, kb * 128:(kb + 1) * 128],
                                    id_bf[0:F, 0:F])
                nc.scalar.activation(muT[:, kb, :], mt[:], AF.Copy)

            # ---- shared conv (mask 3 -> NH), pre-shifted rows -----------
            actv = cst.tile([NH, SR, GW], BF16)
            bord = actv[:, :, 0:1]
            nc.vector.tensor_copy(
                bass.AP(tensor=bord.tensor, offset=bord.offset,
                        ap=[bord.ap[0], [GW, SR], [GW - 1, 2]]),
                zsb[:].rearrange("p (a b) -> p a b", a=SR))
            m3 = mask27[:].rearrange("p (r c) -> p r c", c=GW)
            for a in range(ACH):
                r = 3 * a
                psh = paux.tile([NH, 3, 128], F32, tag="aux")
                nc.tensor.matmul(psh[:], sswT_sb[:], m3[:, r:r + 3, 0:128],
                                 start=True, stop=True)
                nc.scalar.activation(actv[:, r:r + 3, 1:129], psh[:], AF.Relu,
                                     bias=ssb_t[:], scale=1.0)

            # ---- spade gamma/beta lhsT: scale then 9 transposes ---------
            nc.vector.tensor_scalar_mul(sgb[:], sgb[:], om_gba[:])
            spT = cst.tile([128, 9, 128], BF16)
            sgb3 = sgb[:].rearrange("p (l t) -> p l t", t=9)
            for t in range(9):
                pt = paux.tile([128, 128], BF16, tag="aux")
                nc.tensor.transpose(pt[:], sgb3[:, :, t], id_bf[:])
                nc.scalar.activation(spT[:, t, :], pt[:], AF.Copy)

            # ---- region masks part 2: sel = seg * t (DVE) ---------------
            for off, n in segchunks:
                nc.vector.tensor_mul(sel45[:, off:off + n],
                                     sel45[:, off:off + n],
                                     t_sb[:, off:off + n])
            nc.vector.tensor_scalar_mul(actv[:, 0, :], actv[:, 0, :],
                                        hal_t[:, 0:1])
            nc.vector.tensor_scalar_mul(actv[:, SR - 1, :], actv[:, SR - 1, :],
                                        hal_t[:, 1:2])

            # ---- G matmuls (gamma/beta halves pre-scaled via muT) -------
            gps = [gpsp.tile([F, 3, 128], F32, tag="gps", name=f"gps{_g}")
                   for _g in range(3)]
            for kb in range(4):
                for g in range(3):
                    nc.tensor.matmul(gps[g][:], muT[:, kb, :],
                                     wct_sb[:, kb, 3 * g:3 * g + 3, :],
                                     start=(kb == 0), stop=(kb == 3))
            selG = cst.tile([45, 128], BF16)
            gstage = cst.tile([F, 9, 128], BF16)
            for g in range(3):
                nc.scalar.activation(gstage[:, 3 * g:3 * g + 3, :],
                                     gps[g][:], AF.Copy)
            for t in range(9):
                nc.sync.dma_start(out=selG[F * t:F * t + F, :],
                                  in_=gstage[:, t, :])
            # blend gamma/beta halves: selG[:, c] *= (ga if c<64 else ba)
            nc.vector.tensor_mul(selG[:], selG[:], blendT[0:45, :])

            # ---- instance-norm stats (from the resident x plane) --------
            stats_t = cst.tile([C, 32, 6], F32)
            x32 = xb_sb[:].rearrange("c (k n) -> c k n", k=32)
            for k in range(32):
                nc.vector.bn_stats(out=stats_t[:, k, :], in_=x32[:, k, :])
            mv = cst.tile([C, 2], F32)
            nc.vector.bn_aggr(out=mv[:], in_=stats_t[:])
            sd = cst.tile([C, 1], F32)
            nc.scalar.activation(sd[:], mv[:, 1:2], AF.Sqrt, bias=eps_t[:],
                                 scale=1.0)
            rstd = cst.tile([C, 1], F32)
            nc.vector.reciprocal(rstd[:], sd[:])
            nbias = cst.tile([C, 1], F32)
            nc.vector.tensor_mul(nbias[:], mv[:, 0:1], rstd[:])
            nc.vector.tensor_scalar_mul(nbias[:], nbias[:], -1.0)

            # ---- main conv + epilogue (epilogue one chunk behind) -------
            s3 = sel45[:].rearrange("p (r c) -> p r c", c=GW)
            pms = {}

            def conv_chunk(i):
                pm = pmain.tile([128, 4, 128], F32, tag="pm", name=f"pm_{i}")
                pms[i] = pm
                for t in range(9):
                    ty, tx = divmod(t, 3)
                    nc.tensor.matmul(pm[:], spT[:, t, :],
                                     actv[:, 4 * i + ty:4 * i + ty + 4,
                                          tx:tx + 128],
                                     start=(t == 0), stop=False)
                nc.tensor.matmul(pm[:], selG[:], s3[:, 4 * i:4 * i + 4, 0:128],
                                 start=False, stop=True)

            def epi_chunk(i):
                pm = pms.pop(i)
                gb = gbp.tile([128, 4, 128], BF16, tag="gb", name=f"gb_{i}")
                nc.scalar.activation(gb[:], pm[:], AF.Identity,
                                     bias=bias1_t[:], scale=1.0)
                pb = pbtp.tile([64, 4, 128], BF16, tag="pbt", name=f"pb_{i}")
                eng = nc.scalar if i % 2 == 0 else nc.gpsimd
                eng.dma_start(out=pb[:], in_=gb[64:128, :, :])
                xsl = xb_sb[:, i * 512:(i + 1) * 512]
                xnt = xnp.tile([C, 4, 128], F32, tag="xn", name=f"xnt_{i}")
                nc.gpsimd.tensor_scalar(
                    xnt[:].rearrange("p t c -> p (t c)"), xsl,
                    rstd[:], nbias[:], op0=ALU.mult, op1=ALU.add)
                nc.vector.tensor_mul(xnt[:], xnt[:], gb[0:64, :, :])
                nc.vector.tensor_add(xnt[:].rearrange("p t c -> p (t c)"),
                                     xnt[:].rearrange("p t c -> p (t c)"),
                                     pb[:].rearrange("p t c -> p (t c)"))
                if DBG and i == 3:
                    nc.sync.dma_start(out=dbg_gb[:],
                                      in_=gb[:].rearrange("c r w -> c (r w)"))
                nc.sync.dma_start(out=out_d[:, i, :],
                                  in_=xnt[:].rearrange("c r w -> c (r w)"))

            gb_keep = {}
            conv_chunk(0)
            for i in range(1, NCH):
                conv_chunk(i)
                epi_chunk(i - 1)
            epi_chunk(NCH - 1)
            if DBG:
                nc.sync.dma_start(out=dbg_mur[:], in_=mur[:])
                nc.sync.dma_start(out=dbg_selG[:], in_=selG[:])
                nc.sync.dma_start(out=dbg_actv[:],
                                  in_=actv[:, 1:4, :].rearrange("p a b -> p (a b)"))
                nc.sync.dma_start(out=dbg_spT[:],
                                  in_=spT[:, 0:2, :].rearrange("p a b -> p (a b)"))
                nc.sync.dma_start(out=dbg_sel45[:], in_=sel45[:, 2048:2560])
                nc.sync.dma_start(out=dbg_muTg[:],
                                  in_=muT[:].rearrange("p a b -> p (a b)"))
                nc.sync.dma_start(out=dbg_stats[:, 0:1], in_=rstd[:])
                nc.sync.dma_start(out=dbg_stats[:, 1:2], in_=nbias[:])
                nc.sync.dma_start(out=dbg_wct[:],
                                  in_=wct_sb[:, 0, :, :].rearrange("p a b -> p (a b)"))
                nc.sync.dma_start(out=dbg_gst[:],
                                  in_=gstage[:].rearrange("p a b -> p (a b)"))

    nc.finalize()
    return nc


_NC = None


def kernel(**inputs):
    global _NC
    x = np.asarray(inputs["x"], dtype=np.float32)
    segmap = np.asarray(inputs["segmap"], dtype=np.float32)
    codes_vector = np.asarray(inputs["codes_vector"], dtype=np.float32)
    mask = np.asarray(inputs["mask"], dtype=np.float32)
    fc_w = np.asarray(inputs["fc_w"], dtype=np.float32)
    fc_b = np.asarray(inputs["fc_b"], dtype=np.float32)
    conv_gamma_w = np.asarray(inputs["conv_gamma_w"], dtype=np.float32)
    conv_gamma_b = np.asarray(inputs["conv_gamma_b"], dtype=np.float32)
    conv_beta_w = np.asarray(inputs["conv_beta_w"], dtype=np.float32)
    conv_beta_b = np.asarray(inputs["conv_beta_b"], dtype=np.float32)
    spade_shared_w = np.asarray(inputs["spade_shared_w"], dtype=np.float32)
    spade_shared_b = np.asarray(inputs["spade_shared_b"], dtype=np.float32)
    spade_gamma_w = np.asarray(inputs["spade_gamma_w"], dtype=np.float32)
    spade_gamma_b = np.asarray(inputs["spade_gamma_b"], dtype=np.float32)
    spade_beta_w = np.asarray(inputs["spade_beta_w"], dtype=np.float32)
    spade_beta_b = np.asarray(inputs["spade_beta_b"], dtype=np.float32)
    blending_gamma = np.asarray(inputs["blending_gamma"], dtype=np.float32)
    blending_beta = np.asarray(inputs["blending_beta"], dtype=np.float32)

    if _NC is None:
        _NC = _build_nc()

    # fc weights: mu[j,k] = sum_l codes[j,l] * fc_w[j,k,l] -> rhs tiles
    # [128(l_part), 512(k)] per (j, lc): fcwT[j][p, lc*512+k] = fc_w[j,k,lc*128+p]
    fcwT_h = np.ascontiguousarray(
        fc_w.transpose(0, 2, 1).reshape(F, 4, 128, 512).transpose(2, 0, 1, 3)
        .reshape(128, F * 4 * 512)).astype(NPBF)
    # region conv taps: wct[p, lc, t, c] = Wconv[c, lc*128+p, t]
    wc = np.concatenate([conv_gamma_w, conv_beta_w], axis=0)  # [128c, 512, 3, 3]
    wctd_h = np.ascontiguousarray(
        wc.reshape(128, 512, 9).transpose(1, 2, 0)      # [512l, 9t, 128c]
        .reshape(4, 128, 9, 128).transpose(1, 0, 2, 3)  # [128p, 4lc, 9, 128]
        .reshape(128, 4 * 9 * 128)).astype(NPBF)
    # spade gamma/beta stacked, natural layout (device transposes these 9)
    sgbd_h = np.concatenate(
        [spade_gamma_w.reshape(C, NH * 9), spade_beta_w.reshape(C, NH * 9)],
        axis=0).astype(NPBF)
    # shared conv lhsT [27(ty,tx,ic), NH]
    sswT_h = np.ascontiguousarray(
        spade_shared_w.transpose(2, 3, 1, 0).reshape(27, NH)).astype(NPBF)

    shared = {
        "fcwT": fcwT_h,
        "wctd": wctd_h,
        "sgbd": sgbd_h,
        "sswT": sswT_h,
        "fcb": np.ascontiguousarray(fc_b).astype(NPBF),
        "cgb": conv_gamma_b.reshape(C, 1), "cbb": conv_beta_b.reshape(C, 1),
        "sgbb": spade_gamma_b.reshape(C, 1), "sbbb": spade_beta_b.reshape(C, 1),
        "ssb": spade_shared_b.reshape(NH, 1),
        "bg": blending_gamma.reshape(1, 1), "bb": blending_beta.reshape(1, 1),
        "u5": np.kron(np.eye(9, dtype=np.float32),
                      np.tril(np.ones((F, F), np.float32), -1)).astype(NPBF),
        "ident": np.eye(128, dtype=np.float32).astype(NPBF),
    }

    in_maps = []
    for c in range(NCORES):
        b, half = divmod(c, 2)
        h0 = half * ROWS
        segp = np.zeros((F, SEG_SZ), np.float32)
        segp2 = np.zeros((F, SR, GW), np.float32)
        r_lo, r_hi = h0 - 1, h0 + ROWS + 1  # exclusive
        s_lo, s_hi = max(r_lo, 0), min(r_hi, H)
        segp2[:, s_lo - r_lo:s_hi - r_lo, 1:129] = segmap[b, :, s_lo:s_hi, :]
        segp[:, 0:SR * GW] = segp2.reshape(F, -1)
        maskp = np.zeros((3, MASK_SZ), np.float32)
        maskp2 = np.zeros((3, MR, GW), np.float32)
        m_lo, m_hi = h0 - 2, h0 + ROWS + 2
        ms_lo, ms_hi = max(m_lo, 0), min(m_hi, H)
        maskp2[:, ms_lo - m_lo:ms_hi - m_lo, 1:129] = mask[b, :, ms_lo:ms_hi, :]
        maskp[:, 0:MR * GW] = maskp2.reshape(3, -1)
        # x plane rotated so the core's own 64 rows come first: the epilogue
        # reads columns [i*512, (i+1)*512) directly; stats are rotation-
        # invariant.
        xrot = np.roll(x[b].reshape(C, H, W), -h0, axis=1).reshape(C, H * W)
        # block-diagonal codes lhsT [128(p), 4(lc), F(j), F(col)]: column j of
        # the (lc, j) slice holds codes_vector[b, j, lc*128+p], rest zero.
        cT_full = codes_vector[b].T.reshape(4, 128, F).transpose(1, 0, 2)
        codes5 = np.zeros((128, 4, F, F), np.float32)
        for j in range(F):
            codes5[:, :, j, j] = cT_full[:, :, j]
        codes2_h = np.ascontiguousarray(
            codes5.reshape(128, 4 * F * F)).astype(NPBF)
        in_maps.append(dict(
            shared,
            xb=xrot.astype(NPBF),
            hal=np.ones((128, 2), np.float32) * np.array(
                [0.0 if h0 == 0 else 1.0, 0.0 if h0 + ROWS == H else 1.0],
                np.float32)[None, :],
            segg=np.ascontiguousarray(segp).astype(NPBF),
            maskg=np.ascontiguousarray(maskp).astype(NPBF),
            codes2=codes2_h,
        ))

    res = run_bass_kernel_spmd(_NC, in_maps, list(range(NCORES)))

    out = np.empty((B, C, H, W), np.float32)
    for c in range(NCORES):
        b, half = divmod(c, 2)
        h0 = half * ROWS
        out[b, :, h0:h0 + ROWS, :] = res.results[c]["out"].reshape(C, ROWS, W)
    return out
